# revision 46
# baseline (speedup 1.0000x reference)
"""Trainium2 Bass kernel for nn_MultiHeadAttention_87411174408722.

Reference (per batch b, head h; HD == S == 128, E == H*S):
    Q = x@Wq.T+bq, K = x@Wk.T+bk  (V unused by the reference's output)
    M = (Q K^T)/sqrt(HD); A = softmax(M); O = A @ M
    out = concat_h(O) @ Wo.T + bo

Sharding: pure data parallel over batch — 8 batches (1024 tokens) per core.
All layout transforms (x^T, W^T) happen on the host; on-chip everything is
feature-on-partition so matmuls chain without weight transposes.

Schedule (v2): weights stream as host-pre-arranged contiguous per-m column
slabs in consumption order, so the PE starts within ~4us of kernel start and
never waits on a bulk weight load; dummy warm-up matmuls keep the HAM clock
gate at 8/8 through the DMA-paced head.  Wk/bk carry the 1/sqrt(HD) factor
(folded on the host), so sigma lands in PSUM as the reference's M matrix.
Attention per 128x128 block costs 3 PE passes:
    M    = QT_bh^T KT_bh                       (PSUM, [q,k])
    e    = exp(M) -> rowsum d                  (ACT, accum)
    a    = e * (1/d)       per-partition       (DVE)
    at   = a^T                                 (PE transpose)
    O^T  = matmul(lhsT=M, rhs=at):  out[l,q] = sum_k M[k,l] A^T[k,q]
which lands O^T in [l, q] layout directly — no second transpose.  The
PSUM->SBUF drains are split across Scalar and Vector to balance both at
~95% during the K+attention phase.
"""

import numpy as np

import concourse.bass as bass
import concourse.mybir as mybir
import concourse.tile as tile
from concourse.bass import ts
from concourse.bass_utils import run_bass_kernel_spmd
from concourse.masks import make_identity
from concourse.vector_clock import ScopedClock

B, S, E, H = 64, 128, 2048, 16
HD = E // H  # 128
N_CORES = 8
BPC = B // N_CORES  # batches per core
TPC = BPC * S  # tokens per core = 1024
KC = E // 128  # contraction chunks = 16
TB = TPC // 128  # token blocks = 8
EB = E // 512  # output feature slabs = 4
DT = mybir.dt.float16
NP_DT = np.float16
INV_SQRT_HD = 1.0 / float(np.sqrt(HD))

TRACE = False  # test.py sets this for profiled runs

# ---------------------------------------------------------------------------
# Workarounds for this image's walrus sync-wait-slot limit (see waitfix.py):
# the Tile tail Drain and any instruction with many sem waits must have the
# waits split across single/4-wait NOPs.
_counter = [0]


def _chunked_drain_and_barrier(self, tick_clock, wait_clock):
    drain_inst = self.nc.sync.drain()
    wait_clock.add_sem_waits(
        drain_inst.ins, ScopedClock({None: tick_clock.global_clock})
    )
    si = drain_inst.ins.sync_info
    if si is not None and len(si.on_wait) > 1:
        waits = list(si.on_wait)
        del si.on_wait[1:]
        for i in range(1, len(waits)):
            n = self.nc.sync.nop(nofuse=True)
            nsi = n.ins.sync_info
            if nsi is None:
                n.ins.sync_info = mybir.SyncInfo(
                    on_wait=[waits[i]], on_update=[]
                )
            else:
                nsi.on_wait.append(waits[i])

    self.nc.all_engine_barrier()
    assert self.sems is not None
    popped = self.nc._tile_sem_poison_stack.pop()
    assert popped is self._sem_poison
    self.nc.clear_and_free_semaphores(list(self.sems.allocated().values()))
    self.nc.all_engine_barrier()


tile.TileContext._drain_and_barrier = _chunked_drain_and_barrier


def _split_sync_waits(nc, limit=1):
    n_new = 0
    for fn in nc.m.functions:
        for bb in fn.blocks:
            new_list = []
            for inst in bb.instructions:
                si = getattr(inst, "sync_info", None)
                ilim = (
                    1
                    if type(inst).__name__ in ("InstMatmult", "InstLdweights")
                    else limit
                )
                if si is not None and si.on_wait and len(si.on_wait) > ilim:
                    waits = list(si.on_wait)
                    keep = waits[-ilim:]
                    rest = waits[:-ilim]
                    for j in range(0, len(rest), limit):
                        _counter[0] += 1
                        nop = mybir.InstNoOp(
                            name=f"I-wsplit-{_counter[0]}",
                            ins=[],
                            outs=[],
                            sync_info=mybir.SyncInfo(
                                on_wait=list(rest[j : j + limit]), on_update=[]
                            ),
                        )
                        nop.engine = inst.engine
                        new_list.append(nop)
                        n_new += 1
                    del si.on_wait[:]
                    si.on_wait.extend(keep)
                new_list.append(inst)
            bb.instructions[:] = new_list
    return n_new


# ---------------------------------------------------------------------------


def _build():
    nc = bass.Bass(
        "TRN2", target_bir_lowering=False, debug=False, num_devices=N_CORES
    )
    f32 = mybir.dt.float32
    xT_d = nc.dram_tensor("xT", [E, TPC], DT, kind="ExternalInput").ap()
    # WqS/WkS are host-pre-arranged so slab m (= head m's weight columns)
    # is the contiguous [128, 2048] block at rows [m*128, (m+1)*128):
    # element (p, c*128+mm) = W[m*128+mm, c*128+p].
    wqS_d = nc.dram_tensor("WqS", [E, E], DT, kind="ExternalInput").ap()
    wkS_d = nc.dram_tensor("WkS", [E, E], DT, kind="ExternalInput").ap()
    woT_d = nc.dram_tensor("WoT", [E, E], DT, kind="ExternalInput").ap()
    bq_d = nc.dram_tensor("bq", [KC, 128], f32, kind="ExternalInput").ap()
    bk_d = nc.dram_tensor("bk", [KC, 128], f32, kind="ExternalInput").ap()
    bo_d = nc.dram_tensor("bo", [1, E], DT, kind="ExternalInput").ap()
    y_d = nc.dram_tensor("y", [TPC, E], DT, kind="ExternalOutput").ap()

    with tile.TileContext(nc) as tc:
        with (
            tc.tile_pool(name="small", bufs=1) as psmall,
            tc.tile_pool(name="po2t", bufs=1) as po2t,
            tc.tile_pool(name="pws", bufs=5) as pws,
            tc.tile_pool(name="px", bufs=1) as px,
            tc.tile_pool(name="pqk", bufs=1) as pqk,
            tc.tile_pool(name="psProj", bufs=3, space="PSUM") as ps_proj,
            tc.tile_pool(name="psAttn", bufs=2, space="PSUM") as ps_attn,
        ):
            # Warm-up: the PE HAM clock-gate only reaches 2.4 GHz after
            # ~3.4us of sustained busy.  The head of this kernel is
            # DMA-paced (x + first weight slab), which leaves the PE half
            # throttled until ~30us.  Chew on dummy matmuls immediately so
            # the array is warm by the time real chains start.
            warm_t = psmall.tile([128, 512], DT, tag="warm")
            nc.vector.memset(warm_t[:], 0.0)
            warm_ps = ps_proj.tile([128, 512], f32, tag="proj")
            for i in range(12):
                nc.tensor.matmul(
                    warm_ps[:],
                    warm_t[:, 0:128],
                    warm_t[:],
                    start=(i == 0),
                    stop=(i == 11),
                )
            # --- weight slabs, issued in consumption order -----------------
            def w_slab(wS_d, m, name):
                t = pws.tile([128, E], DT, tag="ws", name=name)
                nc.sync.dma_start(t[:], wS_d[ts(m, 128), :])
                return t

            wq_slabs = [w_slab(wqS_d, 0, "wq0")]

            # x chunks, full-width: head DMA is enqueue-bound (~0.6us per
            # dma_start on the Sync queue), so fewer+larger is faster.
            xts = [
                px.tile([128, TPC], DT, tag=f"x{c}", name=f"xt{c}")
                for c in range(KC)
            ]
            for c in range(4):
                nc.sync.dma_start(xts[c][:], xT_d[ts(c, 128), :])
            bq_t = psmall.tile([128, KC], f32, tag="bq")
            nc.sync.dma_start(bq_t[:], bq_d.rearrange("m p -> p m"))
            for c in range(4, KC):
                nc.sync.dma_start(xts[c][:], xT_d[ts(c, 128), :])
            for m in range(1, 7):
                wq_slabs.append(w_slab(wqS_d, m, f"wq{m}"))
            bk_t = psmall.tile([128, KC], f32, tag="bk")
            nc.sync.dma_start(bk_t[:], bk_d.rearrange("m p -> p m"))
            bo_t = psmall.tile([1, E], DT, tag="bo")
            nc.sync.dma_start(bo_t[:], bo_d[:])
            ones_t = psmall.tile([1, 128], DT, tag="ones")
            nc.vector.memset(ones_t[:], 1.0)
            ident = psmall.tile([128, 128], DT, tag="ident")
            make_identity(nc, ident[:])

            for m in range(7, KC):
                wq_slabs.append(w_slab(wqS_d, m, f"wq{m}"))
            wk_slabs = [w_slab(wkS_d, m, f"wk{m}") for m in range(KC)]

            o2t = [
                po2t.tile([128, TPC], DT, tag=f"o{h}", name=f"o2t{h}")
                for h in range(H)
            ]

            def proj_m(slab, m, bias_t, out_t):
                for half in range(2):
                    ps = ps_proj.tile([128, 512], f32, tag="proj")
                    for c in range(KC):
                        nc.tensor.matmul(
                            ps[:],
                            slab[:, ts(c, 128)],
                            xts[c][:, ts(half, 512)],
                            start=(c == 0),
                            stop=(c == KC - 1),
                        )
                    nc.scalar.activation(
                        out_t[:, ts(half, 512)],
                        ps[:],
                        mybir.ActivationFunctionType.Identity,
                        bias=bias_t[:, m : m + 1],
                        scale=1.0,
                    )

            def warm_fill(n):
                # keep the PE busy through DMA-paced stretches so the HAM
                # clock-gate stays at 8/8
                wps = ps_proj.tile([128, 512], f32, tag="proj")
                for i in range(n):
                    nc.tensor.matmul(
                        wps[:],
                        warm_t[:, 0:128],
                        warm_t[:],
                        start=(i == 0),
                        stop=(i == n - 1),
                    )

            # --- Q projection ---------------------------------------------
            qts = []
            for m in range(KC):
                qt_m = pqk.tile([128, TPC], DT, tag=f"q{m}", name=f"qt{m}")
                qts.append(qt_m)
                proj_m(wq_slabs[m], m, bq_t, qt_m)
                if m < 3:
                    warm_fill(10)

            # --- K projection interleaved with attention per head ---------
            for m in range(KC):
                kt_m = pqk.tile([128, TPC], DT, tag="kt", bufs=3, name=f"kt{m}")
                proj_m(wk_slabs[m], m, bk_t, kt_m)
                # blocks processed in quads: the wide [128, 512] PSUM tiles
                # amortize the ~300ns fixed cost of every Scalar/Vector
                # PSUM->SBUF drain over four blocks (those two engines cap
                # this phase, not the PE)
                for bp in range(BPC // 4):
                    b0 = 4 * bp
                    s_ps = ps_attn.tile([128, 512], f32, tag="s", bufs=2)
                    for i in range(4):
                        nc.tensor.matmul(
                            s_ps[:, ts(i, 128)],
                            qts[m][:, ts(b0 + i, 128)],
                            kt_m[:, ts(b0 + i, 128)],
                            start=True,
                            stop=True,
                        )
                    # K weights are pre-scaled by 1/sqrt(HD) on the host, so
                    # s_ps is already the reference's M matrix.
                    e_sb = psmall.tile([128, 512], DT, tag="e", bufs=2)
                    d4_sb = psmall.tile([128, 4], f32, tag="d", bufs=2)
                    for i in range(4):
                        nc.scalar.activation(
                            e_sb[:, ts(i, 128)],
                            s_ps[:, ts(i, 128)],
                            mybir.ActivationFunctionType.Exp,
                            scale=1.0,
                            accum_out=d4_sb[:, i : i + 1],
                        )
                    s_sb = psmall.tile([128, 512], DT, tag="ssb", bufs=2)
                    nc.vector.tensor_copy(s_sb[:], s_ps[:])
                    invd_sb = psmall.tile([128, 4], f32, tag="invd", bufs=2)
                    nc.vector.reciprocal(invd_sb[:], d4_sb[:])
                    a_sb = psmall.tile([128, 512], DT, tag="a", bufs=2)
                    for i in range(4):
                        nc.vector.tensor_scalar_mul(
                            a_sb[:, ts(i, 128)],
                            e_sb[:, ts(i, 128)],
                            invd_sb[:, i : i + 1],
                        )
                    at_ps = ps_attn.tile([128, 512], DT, tag="at", bufs=1)
                    for i in range(4):
                        nc.tensor.transpose(
                            at_ps[:, ts(i, 128)], a_sb[:, ts(i, 128)], ident[:]
                        )
                    at_sb = psmall.tile([128, 512], DT, tag="atsb", bufs=2)
                    nc.vector.tensor_copy(at_sb[:], at_ps[:])
                    o_ps = ps_attn.tile([128, 512], f32, tag="ot", bufs=2)
                    for i in range(4):
                        nc.tensor.matmul(
                            o_ps[:, ts(i, 128)],
                            s_sb[:, ts(i, 128)],
                            at_sb[:, ts(i, 128)],
                            start=True,
                            stop=True,
                        )
                    if bp % 2 == 0:
                        nc.scalar.copy(o2t[m][:, ts(bp, 512)], o_ps[:])
                    else:
                        nc.vector.tensor_copy(o2t[m][:, ts(bp, 512)], o_ps[:])

            # --- output projection, WoT streamed as per-eb column slabs ----
            with tc.tile_pool(name="pwo", bufs=2 * KC) as pwo:
                # bias rows broadcast to all partitions, hoisted out of the
                # eb loop so the per-eb critical path is pure chain work
                bobs = []
                for eb in range(EB):
                    bps = ps_proj.tile([128, 512], f32, tag="proj")
                    nc.tensor.matmul(
                        bps[:],
                        ones_t[:],
                        bo_t[:, ts(eb, 512)],
                        start=True,
                        stop=True,
                    )
                    bob_sb = psmall.tile(
                        [128, 512], f32, tag=f"bob{eb}", name=f"bob{eb}"
                    )
                    nc.vector.tensor_copy(bob_sb[:], bps[:])
                    bobs.append(bob_sb)
                for eb in range(EB):
                    wo_s = [
                        pwo.tile([128, 512], DT, tag="wo", name=f"wo{eb}_{k}")
                        for k in range(KC)
                    ]
                    for k in range(KC):
                        nc.sync.dma_start(
                            wo_s[k][:], woT_d[ts(k, 128), ts(eb, 512)]
                        )
                    bob_sb = bobs[eb]
                    for tb in range(TB):
                        ps = ps_proj.tile([128, 512], f32, tag="proj")
                        for k in range(KC):
                            nc.tensor.matmul(
                                ps[:],
                                o2t[k][:, ts(tb, 128)],
                                wo_s[k][:, :],
                                start=(k == 0),
                                stop=(k == KC - 1),
                            )
                        y_sb = psmall.tile([128, 512], DT, tag="yb", bufs=3)
                        nc.vector.tensor_tensor(
                            y_sb[:], ps[:], bob_sb[:], op=mybir.AluOpType.add
                        )
                        nc.sync.dma_start(y_d[ts(tb, 128), ts(eb, 512)], y_sb[:])

    _split_sync_waits(nc, limit=1)
    return nc


def kernel(x, Wq, bq, Wk, bk, Wv, bv, Wo, bo):
    x = np.asarray(x, dtype=np.float32)
    Wq = np.asarray(Wq, dtype=np.float32)
    Wk = np.asarray(Wk, dtype=np.float32)
    Wo = np.asarray(Wo, dtype=np.float32)
    bq = np.asarray(bq, dtype=np.float32)
    bk = np.asarray(bk, dtype=np.float32)
    bo = np.asarray(bo, dtype=np.float32)

    def slab_layout(W):
        # rows (m p), cols (c mm): element = W[m*128+mm, c*128+p]
        w4 = W.astype(NP_DT).reshape(KC, 128, KC, 128)
        return np.ascontiguousarray(w4.transpose(0, 3, 2, 1).reshape(E, E))

    wqS = slab_layout(Wq)
    # pre-scale K so sigma lands in PSUM already divided by sqrt(HD)
    wkS = slab_layout(Wk * INV_SQRT_HD)
    bk = bk * INV_SQRT_HD
    woT = np.ascontiguousarray(Wo.T.astype(NP_DT))
    bo16 = bo.astype(NP_DT).reshape(1, E)
    bq2 = np.ascontiguousarray(bq.reshape(KC, 128))
    bk2 = np.ascontiguousarray(bk.reshape(KC, 128))

    in_maps = []
    for c in range(N_CORES):
        xs = x[c * BPC : (c + 1) * BPC].reshape(TPC, E)
        xT = np.ascontiguousarray(xs.T.astype(NP_DT))
        in_maps.append(
            {
                "xT": xT,
                "WqS": wqS,
                "WkS": wkS,
                "WoT": woT,
                "bq": bq2,
                "bk": bk2,
                "bo": bo16,
            }
        )

    nc = _build()
    r = run_bass_kernel_spmd(
        nc, in_maps, core_ids=list(range(N_CORES)), trace=TRACE
    )
    if TRACE:
        kernel.last_exec_time_ns = r.exec_time_ns
        kernel.last_results = r
    y = np.concatenate(
        [r.results[c]["y"] for c in range(N_CORES)], axis=0
    ).reshape(B, S, E)
    return np.ascontiguousarray(y, dtype=np.float32)


# revision 47
# speedup vs baseline: 1.0118x; 1.0118x over previous
"""Trainium2 Bass kernel for nn_MultiHeadAttention_87411174408722.

Reference (per batch b, head h; HD == S == 128, E == H*S):
    Q = x@Wq.T+bq, K = x@Wk.T+bk  (V unused by the reference's output)
    M = (Q K^T)/sqrt(HD); A = softmax(M); O = A @ M
    out = concat_h(O) @ Wo.T + bo

Sharding: pure data parallel over batch — 8 batches (1024 tokens) per core.
All layout transforms (x^T, W^T) happen on the host; on-chip everything is
feature-on-partition so matmuls chain without weight transposes.

Schedule (v2): weights stream as host-pre-arranged contiguous per-m column
slabs in consumption order, so the PE starts within ~4us of kernel start and
never waits on a bulk weight load; dummy warm-up matmuls keep the HAM clock
gate at 8/8 through the DMA-paced head.  Wk/bk carry the 1/sqrt(HD) factor
(folded on the host), so sigma lands in PSUM as the reference's M matrix.
Attention per 128x128 block costs 3 PE passes:
    M    = QT_bh^T KT_bh                       (PSUM, [q,k])
    e    = exp(M) -> rowsum d                  (ACT, accum)
    a    = e * (1/d)       per-partition       (DVE)
    at   = a^T                                 (PE transpose)
    O^T  = matmul(lhsT=M, rhs=at):  out[l,q] = sum_k M[k,l] A^T[k,q]
which lands O^T in [l, q] layout directly — no second transpose.  The
PSUM->SBUF drains are split across Scalar and Vector to balance both at
~95% during the K+attention phase.
"""

import numpy as np

import concourse.bass as bass
import concourse.mybir as mybir
import concourse.tile as tile
from concourse.bass import ts
from concourse.bass_utils import run_bass_kernel_spmd
from concourse.masks import make_identity
from concourse.vector_clock import ScopedClock

B, S, E, H = 64, 128, 2048, 16
HD = E // H  # 128
N_CORES = 8
BPC = B // N_CORES  # batches per core
TPC = BPC * S  # tokens per core = 1024
KC = E // 128  # contraction chunks = 16
TB = TPC // 128  # token blocks = 8
EB = E // 512  # output feature slabs = 4
DT = mybir.dt.float16
NP_DT = np.float16
INV_SQRT_HD = 1.0 / float(np.sqrt(HD))

TRACE = False  # test.py sets this for profiled runs

# ---------------------------------------------------------------------------
# Workarounds for this image's walrus sync-wait-slot limit (see waitfix.py):
# the Tile tail Drain and any instruction with many sem waits must have the
# waits split across single/4-wait NOPs.
_counter = [0]


def _chunked_drain_and_barrier(self, tick_clock, wait_clock):
    drain_inst = self.nc.sync.drain()
    wait_clock.add_sem_waits(
        drain_inst.ins, ScopedClock({None: tick_clock.global_clock})
    )
    si = drain_inst.ins.sync_info
    if si is not None and len(si.on_wait) > 1:
        waits = list(si.on_wait)
        del si.on_wait[1:]
        for i in range(1, len(waits)):
            n = self.nc.sync.nop(nofuse=True)
            nsi = n.ins.sync_info
            if nsi is None:
                n.ins.sync_info = mybir.SyncInfo(
                    on_wait=[waits[i]], on_update=[]
                )
            else:
                nsi.on_wait.append(waits[i])

    self.nc.all_engine_barrier()
    assert self.sems is not None
    popped = self.nc._tile_sem_poison_stack.pop()
    assert popped is self._sem_poison
    self.nc.clear_and_free_semaphores(list(self.sems.allocated().values()))
    self.nc.all_engine_barrier()


tile.TileContext._drain_and_barrier = _chunked_drain_and_barrier


def _split_sync_waits(nc, limit=1):
    n_new = 0
    for fn in nc.m.functions:
        for bb in fn.blocks:
            new_list = []
            for inst in bb.instructions:
                si = getattr(inst, "sync_info", None)
                ilim = (
                    1
                    if type(inst).__name__ in ("InstMatmult", "InstLdweights")
                    else limit
                )
                if si is not None and si.on_wait and len(si.on_wait) > ilim:
                    waits = list(si.on_wait)
                    keep = waits[-ilim:]
                    rest = waits[:-ilim]
                    for j in range(0, len(rest), limit):
                        _counter[0] += 1
                        nop = mybir.InstNoOp(
                            name=f"I-wsplit-{_counter[0]}",
                            ins=[],
                            outs=[],
                            sync_info=mybir.SyncInfo(
                                on_wait=list(rest[j : j + limit]), on_update=[]
                            ),
                        )
                        nop.engine = inst.engine
                        new_list.append(nop)
                        n_new += 1
                    del si.on_wait[:]
                    si.on_wait.extend(keep)
                new_list.append(inst)
            bb.instructions[:] = new_list
    return n_new


# ---------------------------------------------------------------------------


def _build():
    nc = bass.Bass(
        "TRN2", target_bir_lowering=False, debug=False, num_devices=N_CORES
    )
    f32 = mybir.dt.float32
    xT_d = nc.dram_tensor("xT", [E, TPC], DT, kind="ExternalInput").ap()
    # WqS/WkS are host-pre-arranged so slab m (= head m's weight columns)
    # is the contiguous [128, 2048] block at rows [m*128, (m+1)*128):
    # element (p, c*128+mm) = W[m*128+mm, c*128+p].
    wqS_d = nc.dram_tensor("WqS", [E, E], DT, kind="ExternalInput").ap()
    wkS_d = nc.dram_tensor("WkS", [E, E], DT, kind="ExternalInput").ap()
    woT_d = nc.dram_tensor("WoT", [E, E], DT, kind="ExternalInput").ap()
    bq_d = nc.dram_tensor("bq", [KC, 128], f32, kind="ExternalInput").ap()
    bk_d = nc.dram_tensor("bk", [KC, 128], f32, kind="ExternalInput").ap()
    bo_d = nc.dram_tensor("bo", [1, E], DT, kind="ExternalInput").ap()
    y_d = nc.dram_tensor("y", [TPC, E], DT, kind="ExternalOutput").ap()

    with tile.TileContext(nc) as tc:
        with (
            tc.tile_pool(name="small", bufs=1) as psmall,
            tc.tile_pool(name="po2t", bufs=1) as po2t,
            tc.tile_pool(name="pws", bufs=5) as pws,
            tc.tile_pool(name="px", bufs=1) as px,
            tc.tile_pool(name="pqk", bufs=1) as pqk,
            tc.tile_pool(name="psProj", bufs=3, space="PSUM") as ps_proj,
            tc.tile_pool(name="psAttn", bufs=2, space="PSUM") as ps_attn,
        ):
            # Warm-up: the PE HAM clock-gate only reaches 2.4 GHz after
            # ~3.4us of sustained busy.  The head of this kernel is
            # DMA-paced (x + first weight slab), which leaves the PE half
            # throttled until ~30us.  Chew on dummy matmuls immediately so
            # the array is warm by the time real chains start.
            warm_t = psmall.tile([128, 512], DT, tag="warm")
            nc.vector.memset(warm_t[:], 0.0)
            warm_ps = ps_proj.tile([128, 512], f32, tag="proj")
            for i in range(12):
                nc.tensor.matmul(
                    warm_ps[:],
                    warm_t[:, 0:128],
                    warm_t[:],
                    start=(i == 0),
                    stop=(i == 11),
                )
            # --- weight slabs, issued in consumption order -----------------
            def w_slab(wS_d, m, name):
                t = pws.tile([128, E], DT, tag="ws", name=name)
                nc.sync.dma_start(t[:], wS_d[ts(m, 128), :])
                return t

            wq_slabs = [w_slab(wqS_d, 0, "wq0")]

            # x chunks, full-width: head DMA is enqueue-bound (~0.6us per
            # dma_start on the Sync queue), so fewer+larger is faster.
            xts = [
                px.tile([128, TPC], DT, tag=f"x{c}", name=f"xt{c}")
                for c in range(KC)
            ]
            for c in range(4):
                nc.sync.dma_start(xts[c][:], xT_d[ts(c, 128), :])
            bq_t = psmall.tile([128, KC], f32, tag="bq")
            nc.sync.dma_start(bq_t[:], bq_d.rearrange("m p -> p m"))
            for c in range(4, KC):
                nc.sync.dma_start(xts[c][:], xT_d[ts(c, 128), :])
            for m in range(1, 7):
                wq_slabs.append(w_slab(wqS_d, m, f"wq{m}"))
            bk_t = psmall.tile([128, KC], f32, tag="bk")
            nc.sync.dma_start(bk_t[:], bk_d.rearrange("m p -> p m"))
            bo_t = psmall.tile([1, E], DT, tag="bo")
            nc.sync.dma_start(bo_t[:], bo_d[:])
            ones_t = psmall.tile([1, 128], DT, tag="ones")
            nc.vector.memset(ones_t[:], 1.0)
            ident = psmall.tile([128, 128], DT, tag="ident")
            make_identity(nc, ident[:])

            for m in range(7, KC):
                wq_slabs.append(w_slab(wqS_d, m, f"wq{m}"))
            wk_slabs = [w_slab(wkS_d, m, f"wk{m}") for m in range(KC)]

            o2t = [
                po2t.tile([128, TPC], DT, tag=f"o{h}", name=f"o2t{h}")
                for h in range(H)
            ]

            def proj_m(slab, m, bias_t, out_t):
                for half in range(2):
                    ps = ps_proj.tile([128, 512], f32, tag="proj")
                    for c in range(KC):
                        nc.tensor.matmul(
                            ps[:],
                            slab[:, ts(c, 128)],
                            xts[c][:, ts(half, 512)],
                            start=(c == 0),
                            stop=(c == KC - 1),
                        )
                    nc.scalar.activation(
                        out_t[:, ts(half, 512)],
                        ps[:],
                        mybir.ActivationFunctionType.Identity,
                        bias=bias_t[:, m : m + 1],
                        scale=1.0,
                    )

            def warm_fill(n):
                # keep the PE busy through DMA-paced stretches so the HAM
                # clock-gate stays at 8/8
                wps = ps_proj.tile([128, 512], f32, tag="proj")
                for i in range(n):
                    nc.tensor.matmul(
                        wps[:],
                        warm_t[:, 0:128],
                        warm_t[:],
                        start=(i == 0),
                        stop=(i == n - 1),
                    )

            # --- Q projection ---------------------------------------------
            qts = []
            for m in range(KC):
                qt_m = pqk.tile([128, TPC], DT, tag=f"q{m}", name=f"qt{m}")
                qts.append(qt_m)
                proj_m(wq_slabs[m], m, bq_t, qt_m)
                if m < 3:
                    warm_fill(10)

            # --- K projection interleaved with attention per head ---------
            for m in range(KC):
                kt_m = pqk.tile([128, TPC], DT, tag="kt", bufs=3, name=f"kt{m}")
                proj_m(wk_slabs[m], m, bk_t, kt_m)
                # blocks processed in pairs: the wide [128, 256] PSUM tiles
                # amortize the ~300ns fixed cost of every Scalar/Vector
                # PSUM->SBUF drain over two blocks (those two engines cap
                # this phase, not the PE)
                for bp in range(BPC // 2):
                    b0 = 2 * bp
                    s_ps = ps_attn.tile([128, 256], f32, tag="s", bufs=2)
                    for i in range(2):
                        nc.tensor.matmul(
                            s_ps[:, ts(i, 128)],
                            qts[m][:, ts(b0 + i, 128)],
                            kt_m[:, ts(b0 + i, 128)],
                            start=True,
                            stop=True,
                        )
                    # K weights are pre-scaled by 1/sqrt(HD) on the host, so
                    # s_ps is already the reference's M matrix.
                    e_sb = psmall.tile([128, 256], DT, tag="e", bufs=3)
                    d4_sb = psmall.tile([128, 2], f32, tag="d", bufs=3)
                    for i in range(2):
                        nc.scalar.activation(
                            e_sb[:, ts(i, 128)],
                            s_ps[:, ts(i, 128)],
                            mybir.ActivationFunctionType.Exp,
                            scale=1.0,
                            accum_out=d4_sb[:, i : i + 1],
                        )
                    s_sb = psmall.tile([128, 256], DT, tag="ssb", bufs=3)
                    nc.vector.tensor_copy(s_sb[:], s_ps[:])
                    invd_sb = psmall.tile([128, 2], f32, tag="invd", bufs=3)
                    nc.vector.reciprocal(invd_sb[:], d4_sb[:])
                    a_sb = psmall.tile([128, 256], DT, tag="a", bufs=3)
                    for i in range(2):
                        nc.vector.tensor_scalar_mul(
                            a_sb[:, ts(i, 128)],
                            e_sb[:, ts(i, 128)],
                            invd_sb[:, i : i + 1],
                        )
                    at_ps = ps_attn.tile([128, 256], DT, tag="at", bufs=1)
                    for i in range(2):
                        nc.tensor.transpose(
                            at_ps[:, ts(i, 128)], a_sb[:, ts(i, 128)], ident[:]
                        )
                    at_sb = psmall.tile([128, 256], DT, tag="atsb", bufs=3)
                    nc.vector.tensor_copy(at_sb[:], at_ps[:])
                    o_ps = ps_attn.tile([128, 256], f32, tag="ot", bufs=2)
                    for i in range(2):
                        nc.tensor.matmul(
                            o_ps[:, ts(i, 128)],
                            s_sb[:, ts(i, 128)],
                            at_sb[:, ts(i, 128)],
                            start=True,
                            stop=True,
                        )
                    if bp % 4 == 0:
                        nc.scalar.copy(o2t[m][:, ts(bp, 256)], o_ps[:])
                    else:
                        nc.vector.tensor_copy(o2t[m][:, ts(bp, 256)], o_ps[:])

            # --- output projection, WoT streamed as per-eb column slabs ----
            with tc.tile_pool(name="pwo", bufs=2 * KC) as pwo:
                # bias rows broadcast to all partitions, hoisted out of the
                # eb loop so the per-eb critical path is pure chain work
                bobs = []
                for eb in range(EB):
                    bps = ps_proj.tile([128, 512], f32, tag="proj")
                    nc.tensor.matmul(
                        bps[:],
                        ones_t[:],
                        bo_t[:, ts(eb, 512)],
                        start=True,
                        stop=True,
                    )
                    bob_sb = psmall.tile(
                        [128, 512], f32, tag=f"bob{eb}", name=f"bob{eb}"
                    )
                    nc.vector.tensor_copy(bob_sb[:], bps[:])
                    bobs.append(bob_sb)
                for eb in range(EB):
                    wo_s = [
                        pwo.tile([128, 512], DT, tag="wo", name=f"wo{eb}_{k}")
                        for k in range(KC)
                    ]
                    for k in range(KC):
                        nc.sync.dma_start(
                            wo_s[k][:], woT_d[ts(k, 128), ts(eb, 512)]
                        )
                    bob_sb = bobs[eb]
                    for tb in range(TB):
                        ps = ps_proj.tile([128, 512], f32, tag="proj")
                        for k in range(KC):
                            nc.tensor.matmul(
                                ps[:],
                                o2t[k][:, ts(tb, 128)],
                                wo_s[k][:, :],
                                start=(k == 0),
                                stop=(k == KC - 1),
                            )
                        y_sb = psmall.tile([128, 512], DT, tag="yb", bufs=3)
                        nc.vector.tensor_tensor(
                            y_sb[:], ps[:], bob_sb[:], op=mybir.AluOpType.add
                        )
                        nc.sync.dma_start(y_d[ts(tb, 128), ts(eb, 512)], y_sb[:])

    _split_sync_waits(nc, limit=1)
    return nc


def kernel(x, Wq, bq, Wk, bk, Wv, bv, Wo, bo):
    x = np.asarray(x, dtype=np.float32)
    Wq = np.asarray(Wq, dtype=np.float32)
    Wk = np.asarray(Wk, dtype=np.float32)
    Wo = np.asarray(Wo, dtype=np.float32)
    bq = np.asarray(bq, dtype=np.float32)
    bk = np.asarray(bk, dtype=np.float32)
    bo = np.asarray(bo, dtype=np.float32)

    def slab_layout(W):
        # rows (m p), cols (c mm): element = W[m*128+mm, c*128+p]
        w4 = W.astype(NP_DT).reshape(KC, 128, KC, 128)
        return np.ascontiguousarray(w4.transpose(0, 3, 2, 1).reshape(E, E))

    wqS = slab_layout(Wq)
    # pre-scale K so sigma lands in PSUM already divided by sqrt(HD)
    wkS = slab_layout(Wk * INV_SQRT_HD)
    bk = bk * INV_SQRT_HD
    woT = np.ascontiguousarray(Wo.T.astype(NP_DT))
    bo16 = bo.astype(NP_DT).reshape(1, E)
    bq2 = np.ascontiguousarray(bq.reshape(KC, 128))
    bk2 = np.ascontiguousarray(bk.reshape(KC, 128))

    in_maps = []
    for c in range(N_CORES):
        xs = x[c * BPC : (c + 1) * BPC].reshape(TPC, E)
        xT = np.ascontiguousarray(xs.T.astype(NP_DT))
        in_maps.append(
            {
                "xT": xT,
                "WqS": wqS,
                "WkS": wkS,
                "WoT": woT,
                "bq": bq2,
                "bk": bk2,
                "bo": bo16,
            }
        )

    nc = _build()
    r = run_bass_kernel_spmd(
        nc, in_maps, core_ids=list(range(N_CORES)), trace=TRACE
    )
    if TRACE:
        kernel.last_exec_time_ns = r.exec_time_ns
        kernel.last_results = r
    y = np.concatenate(
        [r.results[c]["y"] for c in range(N_CORES)], axis=0
    ).reshape(B, S, E)
    return np.ascontiguousarray(y, dtype=np.float32)


# revision 48
# speedup vs baseline: 1.0141x; 1.0023x over previous
"""Trainium2 Bass kernel for nn_MultiHeadAttention_87411174408722.

Reference (per batch b, head h; HD == S == 128, E == H*S):
    Q = x@Wq.T+bq, K = x@Wk.T+bk  (V unused by the reference's output)
    M = (Q K^T)/sqrt(HD); A = softmax(M); O = A @ M
    out = concat_h(O) @ Wo.T + bo

Sharding: pure data parallel over batch — 8 batches (1024 tokens) per core.
All layout transforms (x^T, W^T) happen on the host; on-chip everything is
feature-on-partition so matmuls chain without weight transposes.

Schedule (v2): weights stream as host-pre-arranged contiguous per-m column
slabs in consumption order, so the PE starts within ~4us of kernel start and
never waits on a bulk weight load; dummy warm-up matmuls keep the HAM clock
gate at 8/8 through the DMA-paced head.  Wk/bk carry the 1/sqrt(HD) factor
(folded on the host), so sigma lands in PSUM as the reference's M matrix.
Attention per 128x128 block costs 3 PE passes:
    M    = QT_bh^T KT_bh                       (PSUM, [q,k])
    e    = exp(M) -> rowsum d                  (ACT, accum)
    a    = e * (1/d)       per-partition       (DVE)
    at   = a^T                                 (PE transpose)
    O^T  = matmul(lhsT=M, rhs=at):  out[l,q] = sum_k M[k,l] A^T[k,q]
which lands O^T in [l, q] layout directly — no second transpose.  The
PSUM->SBUF drains are split across Scalar and Vector to balance both at
~95% during the K+attention phase.
"""

import numpy as np

import concourse.bass as bass
import concourse.mybir as mybir
import concourse.tile as tile
from concourse.bass import ts
from concourse.bass_utils import run_bass_kernel_spmd
from concourse.masks import make_identity
from concourse.vector_clock import ScopedClock

B, S, E, H = 64, 128, 2048, 16
HD = E // H  # 128
N_CORES = 8
BPC = B // N_CORES  # batches per core
TPC = BPC * S  # tokens per core = 1024
KC = E // 128  # contraction chunks = 16
TB = TPC // 128  # token blocks = 8
EB = E // 512  # output feature slabs = 4
DT = mybir.dt.float16
NP_DT = np.float16
INV_SQRT_HD = 1.0 / float(np.sqrt(HD))

TRACE = False  # test.py sets this for profiled runs

# ---------------------------------------------------------------------------
# Workarounds for this image's walrus sync-wait-slot limit (see waitfix.py):
# the Tile tail Drain and any instruction with many sem waits must have the
# waits split across single/4-wait NOPs.
_counter = [0]


def _chunked_drain_and_barrier(self, tick_clock, wait_clock):
    drain_inst = self.nc.sync.drain()
    wait_clock.add_sem_waits(
        drain_inst.ins, ScopedClock({None: tick_clock.global_clock})
    )
    si = drain_inst.ins.sync_info
    if si is not None and len(si.on_wait) > 1:
        waits = list(si.on_wait)
        del si.on_wait[1:]
        for i in range(1, len(waits)):
            n = self.nc.sync.nop(nofuse=True)
            nsi = n.ins.sync_info
            if nsi is None:
                n.ins.sync_info = mybir.SyncInfo(
                    on_wait=[waits[i]], on_update=[]
                )
            else:
                nsi.on_wait.append(waits[i])

    self.nc.all_engine_barrier()
    assert self.sems is not None
    popped = self.nc._tile_sem_poison_stack.pop()
    assert popped is self._sem_poison
    self.nc.clear_and_free_semaphores(list(self.sems.allocated().values()))
    self.nc.all_engine_barrier()


tile.TileContext._drain_and_barrier = _chunked_drain_and_barrier


def _split_sync_waits(nc, limit=1):
    n_new = 0
    for fn in nc.m.functions:
        for bb in fn.blocks:
            new_list = []
            for inst in bb.instructions:
                si = getattr(inst, "sync_info", None)
                ilim = (
                    1
                    if type(inst).__name__ in ("InstMatmult", "InstLdweights")
                    else limit
                )
                if si is not None and si.on_wait and len(si.on_wait) > ilim:
                    waits = list(si.on_wait)
                    keep = waits[-ilim:]
                    rest = waits[:-ilim]
                    for j in range(0, len(rest), limit):
                        _counter[0] += 1
                        nop = mybir.InstNoOp(
                            name=f"I-wsplit-{_counter[0]}",
                            ins=[],
                            outs=[],
                            sync_info=mybir.SyncInfo(
                                on_wait=list(rest[j : j + limit]), on_update=[]
                            ),
                        )
                        nop.engine = inst.engine
                        new_list.append(nop)
                        n_new += 1
                    del si.on_wait[:]
                    si.on_wait.extend(keep)
                new_list.append(inst)
            bb.instructions[:] = new_list
    return n_new


# ---------------------------------------------------------------------------


def _build():
    nc = bass.Bass(
        "TRN2", target_bir_lowering=False, debug=False, num_devices=N_CORES
    )
    f32 = mybir.dt.float32
    xT_d = nc.dram_tensor("xT", [E, TPC], DT, kind="ExternalInput").ap()
    # WqS/WkS are host-pre-arranged so slab m (= head m's weight columns)
    # is the contiguous [128, 2048] block at rows [m*128, (m+1)*128):
    # element (p, c*128+mm) = W[m*128+mm, c*128+p].
    wqS_d = nc.dram_tensor("WqS", [E, E], DT, kind="ExternalInput").ap()
    wkS_d = nc.dram_tensor("WkS", [E, E], DT, kind="ExternalInput").ap()
    woT_d = nc.dram_tensor("WoT", [E, E], DT, kind="ExternalInput").ap()
    bq_d = nc.dram_tensor("bq", [KC, 128], f32, kind="ExternalInput").ap()
    bk_d = nc.dram_tensor("bk", [KC, 128], f32, kind="ExternalInput").ap()
    bo_d = nc.dram_tensor("bo", [1, E], DT, kind="ExternalInput").ap()
    y_d = nc.dram_tensor("y", [TPC, E], DT, kind="ExternalOutput").ap()

    with tile.TileContext(nc) as tc:
        with (
            tc.tile_pool(name="small", bufs=1) as psmall,
            tc.tile_pool(name="po2t", bufs=1) as po2t,
            tc.tile_pool(name="pws", bufs=5) as pws,
            tc.tile_pool(name="px", bufs=1) as px,
            tc.tile_pool(name="pqk", bufs=1) as pqk,
            tc.tile_pool(name="psProj", bufs=3, space="PSUM") as ps_proj,
            tc.tile_pool(name="psAttn", bufs=2, space="PSUM") as ps_attn,
        ):
            # Warm-up: the PE HAM clock-gate only reaches 2.4 GHz after
            # ~3.4us of sustained busy.  The head of this kernel is
            # DMA-paced (x + first weight slab), which leaves the PE half
            # throttled until ~30us.  Chew on dummy matmuls immediately so
            # the array is warm by the time real chains start.
            warm_t = psmall.tile([128, 512], DT, tag="warm")
            nc.vector.memset(warm_t[:], 0.0)
            warm_ps = ps_proj.tile([128, 512], f32, tag="proj")
            for i in range(12):
                nc.tensor.matmul(
                    warm_ps[:],
                    warm_t[:, 0:128],
                    warm_t[:],
                    start=(i == 0),
                    stop=(i == 11),
                )
            # --- weight slabs, issued in consumption order -----------------
            def w_slab(wS_d, m, name):
                t = pws.tile([128, E], DT, tag="ws", name=name)
                nc.sync.dma_start(t[:], wS_d[ts(m, 128), :])
                return t

            wq_slabs = [w_slab(wqS_d, 0, "wq0")]

            # x chunks, full-width: head DMA is enqueue-bound (~0.6us per
            # dma_start on the Sync queue), so fewer+larger is faster.
            xts = [
                px.tile([128, TPC], DT, tag=f"x{c}", name=f"xt{c}")
                for c in range(KC)
            ]
            for c in range(4):
                nc.sync.dma_start(xts[c][:], xT_d[ts(c, 128), :])
            bq_t = psmall.tile([128, KC], f32, tag="bq")
            nc.sync.dma_start(bq_t[:], bq_d.rearrange("m p -> p m"))
            for c in range(4, KC):
                nc.sync.dma_start(xts[c][:], xT_d[ts(c, 128), :])
            for m in range(1, 7):
                wq_slabs.append(w_slab(wqS_d, m, f"wq{m}"))
            bk_t = psmall.tile([128, KC], f32, tag="bk")
            nc.sync.dma_start(bk_t[:], bk_d.rearrange("m p -> p m"))
            bo_t = psmall.tile([1, E], DT, tag="bo")
            nc.sync.dma_start(bo_t[:], bo_d[:])
            ones_t = psmall.tile([1, 128], DT, tag="ones")
            nc.vector.memset(ones_t[:], 1.0)
            ident = psmall.tile([128, 128], DT, tag="ident")
            make_identity(nc, ident[:])

            for m in range(7, KC):
                wq_slabs.append(w_slab(wqS_d, m, f"wq{m}"))
            wk_slabs = [w_slab(wkS_d, m, f"wk{m}") for m in range(KC)]

            o2t = [
                po2t.tile([128, TPC], DT, tag=f"o{h}", name=f"o2t{h}")
                for h in range(H)
            ]

            def proj_m(slab, m, bias_t, out_t):
                for half in range(2):
                    ps = ps_proj.tile([128, 512], f32, tag="proj")
                    for c in range(KC):
                        nc.tensor.matmul(
                            ps[:],
                            slab[:, ts(c, 128)],
                            xts[c][:, ts(half, 512)],
                            start=(c == 0),
                            stop=(c == KC - 1),
                        )
                    nc.scalar.activation(
                        out_t[:, ts(half, 512)],
                        ps[:],
                        mybir.ActivationFunctionType.Identity,
                        bias=bias_t[:, m : m + 1],
                        scale=1.0,
                    )

            def warm_fill(n):
                # keep the PE busy through DMA-paced stretches so the HAM
                # clock-gate stays at 8/8
                wps = ps_proj.tile([128, 512], f32, tag="proj")
                for i in range(n):
                    nc.tensor.matmul(
                        wps[:],
                        warm_t[:, 0:128],
                        warm_t[:],
                        start=(i == 0),
                        stop=(i == n - 1),
                    )

            # --- Q projection ---------------------------------------------
            qts = []
            for m in range(KC):
                qt_m = pqk.tile([128, TPC], DT, tag=f"q{m}", name=f"qt{m}")
                qts.append(qt_m)
                proj_m(wq_slabs[m], m, bq_t, qt_m)
                if m < 3:
                    warm_fill(10)

            # --- K projection interleaved with attention per head ---------
            for m in range(KC):
                kt_m = pqk.tile([128, TPC], DT, tag="kt", bufs=3, name=f"kt{m}")
                proj_m(wk_slabs[m], m, bk_t, kt_m)
                # blocks processed in pairs: the wide [128, 256] PSUM tiles
                # amortize the ~300ns fixed cost of every Scalar/Vector
                # PSUM->SBUF drain over two blocks (those two engines cap
                # this phase, not the PE)
                for bp in range(BPC // 2):
                    b0 = 2 * bp
                    s_ps = ps_attn.tile([128, 256], f32, tag="s", bufs=2)
                    for i in range(2):
                        nc.tensor.matmul(
                            s_ps[:, ts(i, 128)],
                            qts[m][:, ts(b0 + i, 128)],
                            kt_m[:, ts(b0 + i, 128)],
                            start=True,
                            stop=True,
                        )
                    # K weights are pre-scaled by 1/sqrt(HD) on the host, so
                    # s_ps is already the reference's M matrix.
                    e_sb = psmall.tile([128, 256], DT, tag="e", bufs=3)
                    d4_sb = psmall.tile([128, 2], f32, tag="d", bufs=3)
                    for i in range(2):
                        nc.scalar.activation(
                            e_sb[:, ts(i, 128)],
                            s_ps[:, ts(i, 128)],
                            mybir.ActivationFunctionType.Exp,
                            scale=1.0,
                            accum_out=d4_sb[:, i : i + 1],
                        )
                    s_sb = psmall.tile([128, 256], DT, tag="ssb", bufs=3)
                    nc.vector.tensor_copy(s_sb[:], s_ps[:])
                    invd_sb = psmall.tile([128, 2], f32, tag="invd", bufs=3)
                    nc.vector.reciprocal(invd_sb[:], d4_sb[:])
                    a_sb = psmall.tile([128, 256], DT, tag="a", bufs=3)
                    for i in range(2):
                        nc.vector.tensor_scalar_mul(
                            a_sb[:, ts(i, 128)],
                            e_sb[:, ts(i, 128)],
                            invd_sb[:, i : i + 1],
                        )
                    at_ps = ps_attn.tile([128, 256], DT, tag="at", bufs=1)
                    for i in range(2):
                        nc.tensor.transpose(
                            at_ps[:, ts(i, 128)], a_sb[:, ts(i, 128)], ident[:]
                        )
                    at_sb = psmall.tile([128, 256], DT, tag="atsb", bufs=3)
                    nc.vector.tensor_copy(at_sb[:], at_ps[:])
                    o_ps = ps_attn.tile([128, 256], f32, tag="ot", bufs=2)
                    for i in range(2):
                        nc.tensor.matmul(
                            o_ps[:, ts(i, 128)],
                            s_sb[:, ts(i, 128)],
                            at_sb[:, ts(i, 128)],
                            start=True,
                            stop=True,
                        )
                    if bp % 2 == 0:
                        nc.scalar.copy(o2t[m][:, ts(bp, 256)], o_ps[:])
                    else:
                        nc.vector.tensor_copy(o2t[m][:, ts(bp, 256)], o_ps[:])

            # --- output projection, WoT streamed as per-eb column slabs ----
            with tc.tile_pool(name="pwo", bufs=2 * KC) as pwo:
                # bias rows broadcast to all partitions, hoisted out of the
                # eb loop so the per-eb critical path is pure chain work
                bobs = []
                for eb in range(EB):
                    bps = ps_proj.tile([128, 512], f32, tag="proj")
                    nc.tensor.matmul(
                        bps[:],
                        ones_t[:],
                        bo_t[:, ts(eb, 512)],
                        start=True,
                        stop=True,
                    )
                    bob_sb = psmall.tile(
                        [128, 512], f32, tag=f"bob{eb}", name=f"bob{eb}"
                    )
                    nc.vector.tensor_copy(bob_sb[:], bps[:])
                    bobs.append(bob_sb)
                for eb in range(EB):
                    wo_s = [
                        pwo.tile([128, 512], DT, tag="wo", name=f"wo{eb}_{k}")
                        for k in range(KC)
                    ]
                    for k in range(KC):
                        nc.sync.dma_start(
                            wo_s[k][:], woT_d[ts(k, 128), ts(eb, 512)]
                        )
                    bob_sb = bobs[eb]
                    for tb in range(TB):
                        ps = ps_proj.tile([128, 512], f32, tag="proj")
                        for k in range(KC):
                            nc.tensor.matmul(
                                ps[:],
                                o2t[k][:, ts(tb, 128)],
                                wo_s[k][:, :],
                                start=(k == 0),
                                stop=(k == KC - 1),
                            )
                        y_sb = psmall.tile([128, 512], DT, tag="yb", bufs=3)
                        nc.vector.tensor_tensor(
                            y_sb[:], ps[:], bob_sb[:], op=mybir.AluOpType.add
                        )
                        nc.sync.dma_start(y_d[ts(tb, 128), ts(eb, 512)], y_sb[:])

    _split_sync_waits(nc, limit=1)
    return nc


def kernel(x, Wq, bq, Wk, bk, Wv, bv, Wo, bo):
    x = np.asarray(x, dtype=np.float32)
    Wq = np.asarray(Wq, dtype=np.float32)
    Wk = np.asarray(Wk, dtype=np.float32)
    Wo = np.asarray(Wo, dtype=np.float32)
    bq = np.asarray(bq, dtype=np.float32)
    bk = np.asarray(bk, dtype=np.float32)
    bo = np.asarray(bo, dtype=np.float32)

    def slab_layout(W):
        # rows (m p), cols (c mm): element = W[m*128+mm, c*128+p]
        w4 = W.astype(NP_DT).reshape(KC, 128, KC, 128)
        return np.ascontiguousarray(w4.transpose(0, 3, 2, 1).reshape(E, E))

    wqS = slab_layout(Wq)
    # pre-scale K so sigma lands in PSUM already divided by sqrt(HD)
    wkS = slab_layout(Wk * INV_SQRT_HD)
    bk = bk * INV_SQRT_HD
    woT = np.ascontiguousarray(Wo.T.astype(NP_DT))
    bo16 = bo.astype(NP_DT).reshape(1, E)
    bq2 = np.ascontiguousarray(bq.reshape(KC, 128))
    bk2 = np.ascontiguousarray(bk.reshape(KC, 128))

    in_maps = []
    for c in range(N_CORES):
        xs = x[c * BPC : (c + 1) * BPC].reshape(TPC, E)
        xT = np.ascontiguousarray(xs.T.astype(NP_DT))
        in_maps.append(
            {
                "xT": xT,
                "WqS": wqS,
                "WkS": wkS,
                "WoT": woT,
                "bq": bq2,
                "bk": bk2,
                "bo": bo16,
            }
        )

    nc = _build()
    r = run_bass_kernel_spmd(
        nc, in_maps, core_ids=list(range(N_CORES)), trace=TRACE
    )
    if TRACE:
        kernel.last_exec_time_ns = r.exec_time_ns
        kernel.last_results = r
    y = np.concatenate(
        [r.results[c]["y"] for c in range(N_CORES)], axis=0
    ).reshape(B, S, E)
    return np.ascontiguousarray(y, dtype=np.float32)
